# revision 1
# baseline (speedup 1.0000x reference)
"""Trainium2 Bass kernel for nn_AdvDiffSolver: 1D advection-diffusion explicit Euler.

y_{t+1}[i] = c0*y[i] + cm*y[i-1] + cp*y[i+1]  (zero-padded boundaries), per-batch coeffs
  alpha = DT*d/DX^2, beta = DT*c/(2*DX);  c0 = 1-2a, cm = a+b, cp = a-b

FIR factorization (2 DVE ops/step instead of 3): L = cm*E- + c0*I + cp*E+
factors exactly as sigma*(1 + s1*E-)(1 + s2*E+) with
  sigma = (c0 + sqrt(c0^2 - 4*cm*cp))/2   (discriminant >= 0.6 for these params)
  s1 = cm/sigma, s2 = cp/sigma.
The device evolves the rescaled state yt = y/sigma^phi within each 16-step
window (sigma^-16 <= ~45, fp32-safe); a tensor_scalar rescale by sigma^16
restores the raw basis at each margin refresh (margin copies fold the scale
into their ACT copy). The per-window sigma^(phi+1) descale of the OUTPUT is
done on the host after the gather (the device stores the scaled basis).

Sharding: pure data parallel, 8 batches per core. Per-core state: one big
rotating tile [128, 10 x 98]: 10 state slots of [16 chunks x 8 batches
partitions, 64-cell core + 17-cell margins]. Margins refresh every H=16
steps via two PE shift-matmuls -> PSUM -> scaled ACT copies; the matmuls
read the unrescaled state so PE starts immediately at step end.
Each step: 2 fused scalar_tensor_tensor (+2 Dirichlet pad memsets) on DVE.
Every 5 steps ONE ACT copy moves 5 states into the [cell x t] accumulation
buffer (strided), so cross-engine traffic is 1/5 per step; the DMA of each
125-step chunk writes 500B-contiguous bursts. Output leaves permuted
[128, 64, T]; host unpermutes + descales.
"""

import numpy as np

B, N, T = 64, 1024, 1000
NCORES = 8
BL = B // NCORES      # 8 batches per core
S = 16                # spatial chunks per sample
CW = N // S           # 64 cells per chunk
M = 17                # margin cells each side
H = 16                # margin refresh period (steps)
W = CW + 2 * M        # 98 tile cols
NB = 10               # state-slot rotation depth (multiple of copy group 5)
CG = 5                # steps per ACT accumulation copy
TC = 125              # time slices per accumulation chunk (8 chunks)
DX = 0.01
DT = 0.01
PACK = CW + 3 + 256   # packed consts: init(64) | s1,s2,sig16 | shd(128) | shu(128)

# chunk s -> partition block blk: s=0 at block 0, s=15 at block 4 (partition 32)
BLK = {}
for s in range(S):
    if s <= 3:
        BLK[s] = s
    elif s == 15:
        BLK[s] = 4
    else:
        BLK[s] = s + 1
INV_BLK = {v: k for k, v in BLK.items()}

_CACHE = {}


def _build(t_total, tc_chunk, h_refresh, m_margin):
    import concourse.bass as bass
    import concourse.bacc as bacc
    import concourse.mybir as mybir
    from concourse import tile

    dt32 = mybir.dt.float32
    add = mybir.AluOpType.add
    mult = mybir.AluOpType.mult
    w = CW + 2 * m_margin

    nc = bacc.Bacc(None, target_bir_lowering=False, debug=False)
    pack_in = nc.dram_tensor("pack", [128, PACK], dt32, kind="ExternalInput")
    out_dram = nc.dram_tensor("out", [128, CW, t_total], dt32, kind="ExternalOutput")

    n_chunks = t_total // tc_chunk
    assert n_chunks * tc_chunk == t_total
    core_l = m_margin
    core_r = m_margin + CW

    with tile.TileContext(nc) as tc:
        with (
            tc.tile_pool(name="state", bufs=1) as sp,
            tc.tile_pool(name="work", bufs=6) as wp,
            tc.tile_pool(name="accp", bufs=2) as ap,
            tc.tile_pool(name="psum", bufs=2, space="PSUM") as pp,
        ):
            ybig = sp.tile([128, NB * w], dt32, name="ybig", tag="ybig")
            yraw = sp.tile([128, w], dt32, name="yraw", tag="yraw")
            consts = sp.tile([128, PACK], dt32, tag="consts")

            nc.sync.dma_start(consts[:], pack_in[:])
            s1t = consts[:, CW:CW + 1]
            s2t = consts[:, CW + 1:CW + 2]
            sg16 = consts[:, CW + 2:CW + 3]
            shd = consts[:, CW + 3:CW + 3 + 128]
            shu = consts[:, CW + 3 + 128:CW + 3 + 256]

            nc.vector.memset(ybig[:], 0.0)
            nc.vector.memset(yraw[:], 0.0)
            nc.vector.tensor_copy(ybig[:, core_l:core_r], consts[:, 0:CW])
            acc = ap.tile([128, CW * tc_chunk], dt32, tag="acc")
            # [p][cell][slot] view of the state tile for the grouped copies
            yv = ybig[:].rearrange("p (nb w) -> p w nb", w=w)
            av = acc[:].rearrange("p (n j) -> p n j", j=tc_chunk)

            for t in range(1, t_total):
                cb = (t - 1) % NB
                nb = t % NB
                co = cb * w   # cur col offset
                no = nb * w   # nxt col offset
                cur = ybig
                if (t - 1) % h_refresh == 0:
                    psl = pp.tile([128, m_margin], dt32, tag="psl")
                    psr = pp.tile([128, m_margin], dt32, tag="psr")
                    # matmuls read the UNRESCALED state so PE starts right at
                    # step end; the sigma^16 rescale rides on the margin copies
                    nc.tensor.matmul(
                        psl[:], shd, ybig[:, co + core_r - m_margin:co + core_r],
                        start=True, stop=True,
                    )
                    nc.tensor.matmul(
                        psr[:], shu, ybig[:, co + core_l:co + core_l + m_margin],
                        start=True, stop=True,
                    )
                    if t > 1:
                        nc.vector.tensor_scalar_mul(
                            yraw[:, core_l:core_r],
                            ybig[:, co + core_l:co + core_r], sg16)
                        nc.scalar.mul(yraw[:, 0:m_margin], psl[:], sg16)
                        nc.scalar.mul(
                            yraw[:, core_r:core_r + m_margin], psr[:], sg16)
                        cur = yraw
                        co = 0
                    else:
                        nc.scalar.copy(ybig[:, 0:m_margin], psl[:])
                        nc.scalar.copy(
                            ybig[:, core_r:core_r + m_margin], psr[:])

                # variable-width window: validity shrinks 1 col/step since
                # the last refresh, so later steps process narrower spans
                phi = (t - 1) % h_refresh
                r = h_refresh - phi
                lo = max(1, core_l - r)
                hi = min(w - 1, core_r + r)
                u = wp.tile([128, w], dt32, tag="u")
                # factored step: u = (1 + s1*E-) yt ; nxt = (1 + s2*E+) u
                nc.vector.scalar_tensor_tensor(
                    u[:, lo:hi + 1], cur[:, co + lo - 1:co + hi], s1t,
                    cur[:, co + lo:co + hi + 1], op0=mult, op1=add)
                nc.vector.scalar_tensor_tensor(
                    ybig[:, no + lo:no + hi], u[:, lo + 1:hi + 1], s2t,
                    u[:, lo:hi], op0=mult, op1=add)
                # Dirichlet clamp: re-zero the innermost pad cell of the two
                # domain-boundary chunks (s=0 at partitions [0,8), s=15 at [32,40))
                nc.vector.memset(ybig[0:8, no + core_l - 1:no + core_l], 0.0)
                nc.vector.memset(ybig[32:40, no + core_r:no + core_r + 1], 0.0)

                j = t % tc_chunk
                if t % CG == CG - 1 or j == tc_chunk - 1:
                    # one ACT copy moves the last CG states (scaled basis)
                    g = CG if t % CG == CG - 1 else tc_chunk % CG
                    sb = (t - g + 1) % NB
                    j0 = j - g + 1
                    nc.scalar.copy(
                        av[:, :, j0:j0 + g],
                        yv[:, core_l:core_r, sb:sb + g],
                    )
                if j == tc_chunk - 1:
                    c = t // tc_chunk
                    dst3 = out_dram[:, :, c * tc_chunk:(c + 1) * tc_chunk]
                    src3 = acc[:].rearrange("p (n j) -> p n j", j=tc_chunk)
                    for k in range(4):
                        nc.sync.dma_start(
                            dst3[:, 16 * k:16 * (k + 1), :],
                            src3[:, 16 * k:16 * (k + 1), :],
                        )
                    if c + 1 < n_chunks:
                        acc = ap.tile([128, CW * tc_chunk], dt32, tag="acc")
                        av = acc[:].rearrange("p (n j) -> p n j", j=tc_chunk)

    nc.finalize()
    return nc


def _coeffs(params):
    d = params[:, 0].astype(np.float64)
    c = params[:, 1].astype(np.float64)
    alpha = DT * d / (DX * DX)
    beta = DT * c / (2.0 * DX)
    c0 = 1.0 - 2.0 * alpha
    cm = alpha + beta
    cp = alpha - beta
    sigma = 0.5 * (c0 + np.sqrt(c0 * c0 - 4.0 * cm * cp))
    return cm, cp, sigma


def _host_prep(init_conds, params):
    """Per-core packed input: permuted init + factored coeffs + shift selectors."""
    cm, cp, sigma = _coeffs(params)
    s1 = (cm / sigma).astype(np.float32)
    s2 = (cp / sigma).astype(np.float32)
    sig16 = (sigma ** H).astype(np.float32)

    # block-major partitions p = BLK[s]*8 + b; shift selectors route chunk
    # neighbors (same batch), domain-boundary chunks source zero.
    shd = np.zeros((128, 128), np.float32)
    shu = np.zeros((128, 128), np.float32)
    for s in range(S):
        for b in range(BL):
            p = BLK[s] * 8 + b
            if s > 0:
                shd[BLK[s - 1] * 8 + b, p] = 1.0
            if s < S - 1:
                shu[BLK[s + 1] * 8 + b, p] = 1.0

    in_maps = []
    for core in range(NCORES):
        sl = slice(core * BL, (core + 1) * BL)
        ic = np.ascontiguousarray(init_conds[sl]).astype(np.float32)
        pack = np.zeros((128, PACK), np.float32)
        icv = ic.reshape(BL, S, CW)
        for s in range(S):
            pack[BLK[s] * 8:BLK[s] * 8 + 8, 0:CW] = icv[:, s, :]
        pack[:, CW] = np.tile(s1[sl], S)
        pack[:, CW + 1] = np.tile(s2[sl], S)
        pack[:, CW + 2] = np.tile(sig16[sl], S)
        pack[:, CW + 3:CW + 3 + 128] = shd
        pack[:, CW + 3 + 128:CW + 3 + 256] = shu
        in_maps.append({"pack": pack})
    return in_maps


def _unpermute(res):
    """[128, 64, T] block-major -> [BL, N, T]."""
    r = res.reshape(S, BL, CW, res.shape[-1])
    out = np.empty((BL, N, res.shape[-1]), res.dtype)
    for s in range(S):
        out[:, s * CW:(s + 1) * CW, :] = r[BLK[s]]
    return out


def kernel(init_conds, params):
    from concourse.bass_utils import run_bass_kernel_spmd

    if "nc" not in _CACHE:
        _CACHE["nc"] = _build(T, TC, H, M)
    nc = _CACHE["nc"]
    params = np.asarray(params)
    in_maps = _host_prep(np.asarray(init_conds), params)
    res = run_bass_kernel_spmd(nc, in_maps, list(range(NCORES)))
    outs = [_unpermute(np.asarray(res.results[c]["out"])) for c in range(NCORES)]
    out = np.concatenate(outs, axis=0)
    # host descale: stored state is y_t / sigma^(((t-1)%16)+1) for t>=1
    _, _, sigma = _coeffs(params)
    tt = np.arange(T)
    expo = np.where(tt == 0, 0, ((tt - 1) % H) + 1).astype(np.float64)
    fac = (sigma[:, None] ** expo[None, :]).astype(np.float32)  # [B, T]
    out *= fac[:, None, :]
    return out



# revision 10
# speedup vs baseline: 1.0050x; 1.0050x over previous
"""Trainium2 Bass kernel for nn_AdvDiffSolver: 1D advection-diffusion explicit Euler.

y_{t+1}[i] = c0*y[i] + cm*y[i-1] + cp*y[i+1]  (zero-padded boundaries), per-batch coeffs
  alpha = DT*d/DX^2, beta = DT*c/(2*DX);  c0 = 1-2a, cm = a+b, cp = a-b

FIR factorization (2 DVE ops/step): L = sigma*(1 + s1*E-)(1 + s2*E+) with
  sigma = (c0 + sqrt(c0^2 - 4*cm*cp))/2, s1 = cm/sigma, s2 = cp/sigma.
The device evolves the rescaled state within each 16-step window; a
tensor_scalar rescale by sigma^16 restores the basis at each margin refresh.
The per-window sigma^(phi+1) descale of the OUTPUT happens on the host.

Sharding: pure data parallel, 8 batches per core.  128 partitions = 16
spatial chunks x 8 batches.  Interior chunks (1..14) hold their 64-cell core
at cols [17,81) with 17-col halo margins both sides, refreshed every H=16
steps via PE shift-matmuls.  The two DOMAIN-BOUNDARY chunks are laid out
shifted so their Dirichlet pad cell falls on a column the step ops never
write: chunk 0 core at [1,65) (pad col 0; STT2 writes cols >= lo >= 1) and
chunk 15 core at [33,97) (pad col 97; STT2 writes cols < hi <= 97).  The pad
columns stay zero from init, so NO per-step boundary memsets are needed --
each step is exactly 2 fused scalar_tensor_tensor DVE ops.  Stale data
outside a boundary chunk's valid span decays inward 1 col/step and never
reaches the core within a refresh window.

Every 5 steps ONE ACT copy moves 5 states (union cols [1,97)) into the
[cell x t] accumulation buffer; each 125-step chunk is DMAd out with
per-chunk-group cell offsets so HBM only carries the 64 core cells.
Output leaves permuted [128, 64, T]; host unpermutes + descales.
"""

import numpy as np

B, N, T = 64, 1024, 1000
NCORES = 8
BL = B // NCORES      # 8 batches per core
S = 16                # spatial chunks per sample
CW = N // S           # 64 cells per chunk
M = 17                # margin cells each side (interior chunks)
H = 16                # margin refresh period (steps)
W = CW + 2 * M        # 98 tile cols
NB = 10               # state-slot rotation depth (multiple of copy group 5)
CG = 5                # steps per ACT accumulation copy
TC = 125              # time slices per accumulation chunk (8 chunks)
DX = 0.01
DT = 0.01
AC = W - 2            # 96: accumulated cols [1,97)
# packed consts: init(96) | s1,s2,sig16 | shd_{i,b,s} | shu_{i,b,s}
PACK = AC + 3 + 6 * 128

# chunk s -> partition block (engine partition windows must start 32-aligned,
# so the boundary chunks sit at blocks 0 and 4: bases 0 and 32)
BLK = {}
for s in range(S):
    if s <= 3:
        BLK[s] = s
    elif s == 15:
        BLK[s] = 4
    else:
        BLK[s] = s + 1
# core column offset per chunk: boundary chunks shifted so the Dirichlet pad
# lands at col 0 (chunk 0) / col 97 (chunk 15)
C0OF = {s: (1 if s == 0 else (33 if s == 15 else M)) for s in range(S)}

_CACHE = {}


def _build(t_total, tc_chunk, h_refresh, m_margin):
    import concourse.bass as bass
    import concourse.bacc as bacc
    import concourse.mybir as mybir
    from concourse import tile

    dt32 = mybir.dt.float32
    add = mybir.AluOpType.add
    mult = mybir.AluOpType.mult
    w = CW + 2 * m_margin

    nc = bacc.Bacc(None, target_bir_lowering=False, debug=False)
    pack_in = nc.dram_tensor("pack", [128, PACK], dt32, kind="ExternalInput")
    out_dram = nc.dram_tensor("out", [128, CW, t_total], dt32, kind="ExternalOutput")

    n_chunks = t_total // tc_chunk
    assert n_chunks * tc_chunk == t_total
    core_l = m_margin          # 17 (interior window anchor)
    core_r = m_margin + CW     # 81

    with tile.TileContext(nc) as tc:
        with (
            tc.tile_pool(name="state", bufs=1) as sp,
            tc.tile_pool(name="work", bufs=6) as wp,
            tc.tile_pool(name="accp", bufs=2) as ap,
            tc.tile_pool(name="psum", bufs=2, space="PSUM") as pp,
        ):
            ybig = sp.tile([128, NB * w], dt32, name="ybig", tag="ybig")
            yraw = sp.tile([128, w], dt32, name="yraw", tag="yraw")
            consts = sp.tile([128, PACK], dt32, tag="consts")

            nc.sync.dma_start(consts[:], pack_in[:])
            s1t = consts[:, AC:AC + 1]
            s2t = consts[:, AC + 1:AC + 2]
            sg16 = consts[:, AC + 2:AC + 3]
            o = AC + 3
            shd_i = consts[:, o:o + 128]
            shd_b = consts[:, o + 128:o + 256]
            shd_s = consts[:, o + 256:o + 384]
            shu_i = consts[:, o + 384:o + 512]
            shu_b = consts[:, o + 512:o + 640]
            shu_s = consts[:, o + 640:o + 768]

            nc.vector.memset(ybig[:], 0.0)
            nc.vector.memset(yraw[:], 0.0)
            # init state (host pre-shifted per chunk group) into slot 0
            nc.vector.tensor_copy(ybig[:, 1:1 + AC], consts[:, 0:AC])
            acc = ap.tile([128, AC * tc_chunk], dt32, tag="acc")
            # [p][cell][slot] view of the state tile for the grouped copies
            yv = ybig[:].rearrange("p (nb w) -> p w nb", w=w)
            av = acc[:].rearrange("p (n j) -> p n j", j=tc_chunk)

            for t in range(1, t_total):
                cb = (t - 1) % NB
                nb = t % NB
                co = cb * w   # cur col offset
                no = nb * w   # nxt col offset
                cur = ybig
                if (t - 1) % h_refresh == 0:
                    psl = pp.tile([128, m_margin], dt32, tag="psl")
                    psr = pp.tile([128, m_margin], dt32, tag="psr")
                    # halo refresh: partition-shift matmuls read the
                    # UNRESCALED state.  Three col slices accumulate into each
                    # PSUM tile: interior sources, boundary-chunk sources, and
                    # identity self-routes that make the full-128 std copies
                    # write-safe over the shifted boundary cores.
                    nc.tensor.matmul(
                        psl[:], shd_i, ybig[:, co + 64:co + 81],
                        start=True, stop=False,
                    )
                    nc.tensor.matmul(
                        psl[:], shd_b, ybig[:, co + 48:co + 65],
                        start=False, stop=False,
                    )
                    nc.tensor.matmul(
                        psl[:], shd_s, ybig[:, co + 0:co + 17],
                        start=False, stop=True,
                    )
                    nc.tensor.matmul(
                        psr[:], shu_i, ybig[:, co + 17:co + 34],
                        start=True, stop=False,
                    )
                    nc.tensor.matmul(
                        psr[:], shu_b, ybig[:, co + 33:co + 50],
                        start=False, stop=False,
                    )
                    nc.tensor.matmul(
                        psr[:], shu_s, ybig[:, co + 81:co + 98],
                        start=False, stop=True,
                    )
                    if t > 1:
                        nc.vector.tensor_scalar_mul(
                            yraw[:, 1:97], ybig[:, co + 1:co + 97], sg16)
                        # std copies first (full 128), then the two aligned
                        # special-window copies overwrite the boundary cols
                        nc.scalar.mul(yraw[:, 0:17], psl[:], sg16)
                        nc.scalar.mul(yraw[:, 81:98], psr[:], sg16)
                        nc.scalar.mul(yraw[32:40, 16:33], psl[32:40, :],
                                      sg16[32:40])
                        nc.scalar.mul(yraw[0:8, 65:82], psr[0:8, :],
                                      sg16[0:8])
                        cur = yraw
                        co = 0
                    else:
                        nc.scalar.copy(ybig[:, 0:17], psl[:])
                        nc.scalar.copy(ybig[:, 81:98], psr[:])
                        nc.scalar.copy(ybig[32:40, 16:33], psl[32:40, :])
                        nc.scalar.copy(ybig[0:8, 65:82], psr[0:8, :])

                # fixed full-width window: the shifted boundary cores sit at
                # cols 1 and 96, so every step must write [1,97).  Expired
                # interior margin cells get garbage — harmless by validity
                # accounting (reads near the core stay in-validity).
                lo = 1
                hi = w - 1
                u = wp.tile([128, w], dt32, tag="u")
                # factored step: u = (1 + s1*E-) yt ; nxt = (1 + s2*E+) u
                nc.vector.scalar_tensor_tensor(
                    u[:, lo:hi + 1], cur[:, co + lo - 1:co + hi], s1t,
                    cur[:, co + lo:co + hi + 1], op0=mult, op1=add)
                nc.vector.scalar_tensor_tensor(
                    ybig[:, no + lo:no + hi], u[:, lo + 1:hi + 1], s2t,
                    u[:, lo:hi], op0=mult, op1=add)

                j = t % tc_chunk
                if t % CG == CG - 1 or j == tc_chunk - 1:
                    # one ACT copy moves the last CG states (scaled basis),
                    # union cols [1,97) covering all chunk-group cores
                    g = CG if t % CG == CG - 1 else tc_chunk % CG
                    sb = (t - g + 1) % NB
                    j0 = j - g + 1
                    nc.scalar.copy(
                        av[:, :, j0:j0 + g],
                        yv[:, 1:1 + AC, sb:sb + g],
                    )
                if j == tc_chunk - 1:
                    c = t // tc_chunk
                    dst3 = out_dram[:, :, c * tc_chunk:(c + 1) * tc_chunk]
                    src3 = acc[:].rearrange("p (n j) -> p n j", j=tc_chunk)
                    # per-group cell offsets: interior cores at acc idx
                    # [16,80), chunk 0 (parts 0:8) at [0,64), chunk 15
                    # (parts 32:40) at [32,96)
                    nc.sync.dma_start(dst3[0:8, :, :], src3[0:8, 0:64, :])
                    nc.sync.dma_start(dst3[8:32, :, :], src3[8:32, 16:80, :])
                    nc.sync.dma_start(dst3[32:40, :, :],
                                      src3[32:40, 32:96, :])
                    for k in range(2):
                        nc.sync.dma_start(
                            dst3[40:128, 32 * k:32 * (k + 1), :],
                            src3[40:128, 16 + 32 * k:48 + 32 * k, :],
                        )
                    if c + 1 < n_chunks:
                        acc = ap.tile([128, AC * tc_chunk], dt32, tag="acc")
                        av = acc[:].rearrange("p (n j) -> p n j", j=tc_chunk)

    nc.finalize()
    return nc


def _coeffs(params):
    d = params[:, 0].astype(np.float64)
    c = params[:, 1].astype(np.float64)
    alpha = DT * d / (DX * DX)
    beta = DT * c / (2.0 * DX)
    c0 = 1.0 - 2.0 * alpha
    cm = alpha + beta
    cp = alpha - beta
    sigma = 0.5 * (c0 + np.sqrt(c0 * c0 - 4.0 * cm * cp))
    return cm, cp, sigma


def _host_prep(init_conds, params):
    """Per-core packed input: shifted init + factored coeffs + shift selectors."""
    cm, cp, sigma = _coeffs(params)
    s1 = (cm / sigma).astype(np.float32)
    s2 = (cp / sigma).astype(np.float32)
    sig16 = (sigma ** H).astype(np.float32)

    # partition p = BLK[s]*8 + b; selector matrices route chunk halos
    # (same batch).  Interior sources use the standard col slices; the
    # boundary-chunk sources get their own matrices (shifted col slices);
    # the _s identity matrices route each boundary chunk's own cells so the
    # full-128 std margin copies rewrite them in place (write-safe).
    shd_i = np.zeros((128, 128), np.float32)
    shd_b = np.zeros((128, 128), np.float32)
    shd_s = np.zeros((128, 128), np.float32)
    shu_i = np.zeros((128, 128), np.float32)
    shu_b = np.zeros((128, 128), np.float32)
    shu_s = np.zeros((128, 128), np.float32)
    for b in range(BL):
        for s in range(1, S):       # dest s left margin <- source s-1
            src, dst = BLK[s - 1] * 8 + b, BLK[s] * 8 + b
            (shd_b if s == 1 else shd_i)[src, dst] = 1.0
        for s in range(S - 1):      # dest s right margin <- source s+1
            src, dst = BLK[s + 1] * 8 + b, BLK[s] * 8 + b
            (shu_b if s == S - 2 else shu_i)[src, dst] = 1.0
        p0 = BLK[0] * 8 + b         # chunk 0: psl self-route, cols [0,17)
        shd_s[p0, p0] = 1.0
        p15 = BLK[15] * 8 + b       # chunk 15: psr self-route, cols [81,98)
        shu_s[p15, p15] = 1.0

    in_maps = []
    for core in range(NCORES):
        sl = slice(core * BL, (core + 1) * BL)
        ic = np.ascontiguousarray(init_conds[sl]).astype(np.float32)
        pack = np.zeros((128, PACK), np.float32)
        icv = ic.reshape(BL, S, CW)
        for s in range(S):
            f0 = C0OF[s] - 1   # init-field idx of core cell 0 (state col-1)
            pack[BLK[s] * 8:BLK[s] * 8 + 8, f0:f0 + CW] = icv[:, s, :]
        pack[:, AC] = np.tile(s1[sl], S)
        pack[:, AC + 1] = np.tile(s2[sl], S)
        pack[:, AC + 2] = np.tile(sig16[sl], S)
        o = AC + 3
        pack[:, o:o + 128] = shd_i
        pack[:, o + 128:o + 256] = shd_b
        pack[:, o + 256:o + 384] = shd_s
        pack[:, o + 384:o + 512] = shu_i
        pack[:, o + 512:o + 640] = shu_b
        pack[:, o + 640:o + 768] = shu_s
        in_maps.append({"pack": pack})
    return in_maps


def _unpermute(res):
    """[128, 64, T] block-major -> [BL, N, T]."""
    r = res.reshape(S, BL, CW, res.shape[-1])
    out = np.empty((BL, N, res.shape[-1]), res.dtype)
    for s in range(S):
        out[:, s * CW:(s + 1) * CW, :] = r[BLK[s]]
    return out


def kernel(init_conds, params):
    from concourse.bass_utils import run_bass_kernel_spmd

    if "nc" not in _CACHE:
        _CACHE["nc"] = _build(T, TC, H, M)
    nc = _CACHE["nc"]
    params = np.asarray(params)
    in_maps = _host_prep(np.asarray(init_conds), params)
    res = run_bass_kernel_spmd(nc, in_maps, list(range(NCORES)))
    outs = [_unpermute(np.asarray(res.results[c]["out"])) for c in range(NCORES)]
    out = np.concatenate(outs, axis=0)
    # host descale: stored state is y_t / sigma^(((t-1)%16)+1) for t>=1
    _, _, sigma = _coeffs(params)
    tt = np.arange(T)
    expo = np.where(tt == 0, 0, ((tt - 1) % H) + 1).astype(np.float64)
    fac = (sigma[:, None] ** expo[None, :]).astype(np.float32)  # [B, T]
    out *= fac[:, None, :]
    return out
